# revision 1
# baseline (speedup 1.0000x reference)
"""Windowed attention with dynamic position bias — 8-core data-parallel kernel.

Strategy: data-parallel over the batch axis (8 batches -> 8 NeuronCores).
Each core runs the full per-image computation (LN -> window partition ->
qkv -> windowed attention with DPB bias -> out projection -> un-window).
Inputs are full (unsharded); we shard x over batch inside, replicate weights,
and gather the full output.
"""

import numpy as np
import jax
import jax.numpy as jnp
from functools import partial

WSZ = 8
HEADS = 8
DIM_HEAD = 32
DIM = 256
SCALE = DIM_HEAD ** -0.5

N_CORES = 8


def _ln(x, g, b, eps):
    m = jnp.mean(x, axis=-1, keepdims=True)
    v = jnp.var(x, axis=-1, keepdims=True)
    return (x - m) / jnp.sqrt(v + eps) * g + b


def _dpb(rel, w1, b1, g1, bb1, w2, b2, g2, bb2, w3, b3, g3, bb3, w4, b4):
    h = jax.nn.relu(_ln(rel @ w1 + b1, g1, bb1, 1e-3))
    h = jax.nn.relu(_ln(h @ w2 + b2, g2, bb2, 1e-3))
    h = jax.nn.relu(_ln(h @ w3 + b3, g3, bb3, 1e-3))
    return (h @ w4 + b4)[..., 0]


def _shard_fn(x, norm_g, norm_b, w_qkv, w_out, b_out,
              dpb_w1, dpb_b1, dpb_ln1_g, dpb_ln1_b,
              dpb_w2, dpb_b2, dpb_ln2_g, dpb_ln2_b,
              dpb_w3, dpb_b3, dpb_ln3_g, dpb_ln3_b,
              dpb_w4, dpb_b4):
    # x: (Bs, 128, 128, 256) local shard
    B, H, W, D = x.shape
    wsz = WSZ
    nh, nw = H // wsz, W // wsz
    s = wsz * wsz

    xn = _ln(x, norm_g, norm_b, 1e-5)
    xw = xn.reshape(B, nh, wsz, nw, wsz, D).transpose(0, 1, 3, 2, 4, 5)
    xw = xw.reshape(B * nh * nw, s, D)

    qkv = xw @ w_qkv
    q, k, v = jnp.split(qkv, 3, axis=-1)
    split = lambda t: t.reshape(t.shape[0], s, HEADS, DIM_HEAD).transpose(0, 2, 1, 3)
    q, k, v = split(q), split(k), split(v)

    sim = jnp.einsum('bhid,bhjd->bhij', q * SCALE, k)

    pos = jnp.arange(-wsz, wsz + 1)
    rel = jnp.stack(jnp.meshgrid(pos, pos, indexing='ij'), axis=-1)
    rel = rel.reshape(-1, 2).astype(x.dtype)
    biases = _dpb(rel, dpb_w1, dpb_b1, dpb_ln1_g, dpb_ln1_b,
                  dpb_w2, dpb_b2, dpb_ln2_g, dpb_ln2_b,
                  dpb_w3, dpb_b3, dpb_ln3_g, dpb_ln3_b,
                  dpb_w4, dpb_b4)
    p = jnp.arange(wsz)
    g = jnp.stack(jnp.meshgrid(p, p, indexing='ij'), axis=-1).reshape(-1, 2)
    rp = g[:, None] - g[None, :] + wsz - 1
    idx = rp[..., 0] * (2 * wsz - 1) + rp[..., 1]
    sim = sim + biases[idx]

    attn = jax.nn.softmax(sim, axis=-1)
    out = jnp.einsum('bhij,bhjd->bhid', attn, v)
    out = out.transpose(0, 2, 1, 3).reshape(B * nh * nw, s, HEADS * DIM_HEAD)
    out = out @ w_out + b_out
    out = out.reshape(B, nh, nw, wsz, wsz, D).transpose(0, 1, 3, 2, 4, 5)
    out = out.reshape(B, H, W, D)
    return out


_PMAPPED = None


def _get_pmapped():
    global _PMAPPED
    if _PMAPPED is None:
        in_axes = tuple([0] + [None] * 19)
        _PMAPPED = jax.pmap(_shard_fn, in_axes=in_axes, devices=jax.devices()[:N_CORES])
    return _PMAPPED


def kernel(**inputs):
    x = np.asarray(inputs['x'])
    B = x.shape[0]
    assert B % N_CORES == 0
    per = B // N_CORES
    # shard batch across cores: (8, per, H, W, D)
    xs = x.reshape(N_CORES, per, *x.shape[1:])

    names = ['norm_g', 'norm_b', 'w_qkv', 'w_out', 'b_out',
             'dpb_w1', 'dpb_b1', 'dpb_ln1_g', 'dpb_ln1_b',
             'dpb_w2', 'dpb_b2', 'dpb_ln2_g', 'dpb_ln2_b',
             'dpb_w3', 'dpb_b3', 'dpb_ln3_g', 'dpb_ln3_b',
             'dpb_w4', 'dpb_b4']
    rest = [np.asarray(inputs[n]) for n in names]

    fn = _get_pmapped()
    out = fn(xs, *rest)
    out = np.asarray(out)  # (8, per, H, W, D)
    out = out.reshape(B, *x.shape[1:3], DIM).astype(np.float32)
    return out


if __name__ == '__main__':
    # smoke test with random inputs
    rng = np.random.default_rng(0)
    d4 = DIM // 4
    ins = {
        'x': rng.standard_normal((8, 128, 128, DIM), dtype=np.float32),
        'norm_g': np.ones(DIM, np.float32), 'norm_b': np.zeros(DIM, np.float32),
        'w_qkv': (rng.standard_normal((DIM, 3 * HEADS * DIM_HEAD)) * 0.02).astype(np.float32),
        'w_out': (rng.standard_normal((HEADS * DIM_HEAD, DIM)) * 0.02).astype(np.float32),
        'b_out': np.zeros(DIM, np.float32),
        'dpb_w1': (rng.standard_normal((2, d4)) * 0.02).astype(np.float32),
        'dpb_b1': np.zeros(d4, np.float32),
        'dpb_ln1_g': np.ones(d4, np.float32), 'dpb_ln1_b': np.zeros(d4, np.float32),
        'dpb_w2': (rng.standard_normal((d4, d4)) * 0.02).astype(np.float32),
        'dpb_b2': np.zeros(d4, np.float32),
        'dpb_ln2_g': np.ones(d4, np.float32), 'dpb_ln2_b': np.zeros(d4, np.float32),
        'dpb_w3': (rng.standard_normal((d4, d4)) * 0.02).astype(np.float32),
        'dpb_b3': np.zeros(d4, np.float32),
        'dpb_ln3_g': np.ones(d4, np.float32), 'dpb_ln3_b': np.zeros(d4, np.float32),
        'dpb_w4': (rng.standard_normal((d4, 1)) * 0.02).astype(np.float32),
        'dpb_b4': np.zeros(1, np.float32),
    }
    out = kernel(**ins)
    print('out shape', out.shape, out.dtype, 'finite:', np.isfinite(out).all())
